# revision 1
# baseline (speedup 1.0000x reference)
"""Deformable-attention block (nn_DCAB) as a Bass/Tile kernel on 8 TRN2 cores.

Sharding: core c = (b, q4): b = c//4, query-quarter q4 = c%4 (1024 queries),
all 4 heads per core. No collectives.

Pipeline per core (query-major tap math):
  V_l = 1x1conv(f_l) cell-major bf16; Q = LN(S^T qw^T + qb) f32 query-major;
  Q^T via PE transpose; off/wgt matmul; tanh positions on ACT; robust floor;
  softmax over (l,m); duplicate-tap merge on DVE; per-(h,l) one-hot A rows
  built by GPSIMD local_scatter; A^T via SBUF-source dma_gather transpose;
  sampling = PSUM-accumulated bf16 matmuls against V; out_proj; LN; FFN
  (fc1 feature-major with fused Gelu+bias, fc2 query-major); residuals.
"""
import numpy as np
import ml_dtypes

P = 128
C = 256
NQ = 1024
L_LEV, M_PTS = 3, 4
DIMS = [64, 32, 16]
HWs = [4096, 1024, 256]
KT = [32, 8, 2]
N_CORES = 8

_CACHE = {}
SKIP = set()


def _build():
    import concourse.bacc as bacc
    import concourse.mybir as mybir
    import concourse.tile as tile
    from concourse.masks import make_identity
    import contextlib

    dt = mybir.dt
    Alu = mybir.AluOpType
    Act = mybir.ActivationFunctionType

    nc = bacc.Bacc("TRN2", target_bir_lowering=False)

    def din(name, shape, dty=dt.float32):
        return nc.dram_tensor(name, shape, dty, kind="ExternalInput")

    s = din("s", [P, 2, NQ])
    sT = din("sT", [P, 8, C])
    f0 = din("f0", [P, 32, 2, P], dt.bfloat16)
    f1 = din("f1", [P, 8, 4, P], dt.bfloat16)
    f2 = din("f2", [P, 2, 8, P], dt.bfloat16)
    FB = [8, 16, 512, 256, 288, 144, 768, 256, 8, 256, 1024, 96, 96, 48]
    fblob = din("fblob", [P, sum(FB)])
    BB = [512, 1024, 2048]
    bblob = din("bblob", [P, sum(BB)], dt.bfloat16)
    lblob = din("lblob", [P, 6144], dt.bfloat16)
    gidx = din("gidx", [P, 32], dt.int16)     # wrapped-16 identity for 512 tokens
    out = nc.dram_tensor("out", [P, 8, C], dt.float32, kind="ExternalOutput")

    with tile.TileContext(nc) as tc:
        ctx = contextlib.ExitStack()
        with ctx:
            wp = ctx.enter_context(tc.tile_pool(name="wp", bufs=1))
            sp = ctx.enter_context(tc.tile_pool(name="sp", bufs=1))
            tp = ctx.enter_context(tc.tile_pool(name="tp", bufs=1))
            ap_ = ctx.enter_context(tc.tile_pool(name="ap", bufs=2))
            atp = ctx.enter_context(tc.tile_pool(name="atp", bufs=2))
            fp = ctx.enter_context(tc.tile_pool(name="fp", bufs=2))
            qp = ctx.enter_context(tc.tile_pool(name="qp", bufs=2))
            mp = ctx.enter_context(tc.tile_pool(name="mp", bufs=1))
            pp = ctx.enter_context(tc.tile_pool(name="pp", bufs=8, space="PSUM"))

            def ld(pool, ap, name=None):
                t = pool.tile(list(ap.shape), ap.dtype, tag=name or ap.name)
                nc.sync.dma_start(t[:], ap[:])
                return t

            fb_t = ld(wp, fblob)
            bb_t = ld(wp, bblob)
            offs = np.cumsum([0] + FB).tolist()
            fbv = [fb_t[:, offs[i]:offs[i + 1]] for i in range(len(FB))]
            simq_t = fbv[0]
            refq_t = fbv[1].rearrange("p (a b) -> p a b", b=2)
            qwT_t = fbv[2].rearrange("p (k o) -> p k o", k=2)
            qbrep_t = fbv[3]
            owT_t = fbv[4].rearrange("p (k o) -> p k o", k=2)
            owbrep_t = fbv[5]
            vbrep_t = [fbv[6][:, i * C:(i + 1) * C] for i in range(3)]
            outbrep_t = fbv[7]
            fc1b_t = fbv[8]
            fc2brep_t = fbv[9]
            lnqg_t = fbv[10][:, 0:C]; lnqb_t = fbv[10][:, C:2 * C]
            lnog_t = fbv[10][:, 2 * C:3 * C]; lnob_t = fbv[10][:, 3 * C:4 * C]
            screp_t = fbv[11]; wm2rep_t = fbv[12]; wlrep_t = fbv[13]
            boffs = np.cumsum([0] + BB).tolist()
            bbv = [bb_t[:, boffs[i]:boffs[i + 1]] for i in range(len(BB))]
            vwT_t = [bbv[0].rearrange("p (k o) -> p k o", k=2),
                     bbv[1].rearrange("p (k o) -> p k o", k=4),
                     bbv[2].rearrange("p (k o) -> p k o", k=8)]
            gidx_t = ld(wp, gidx)

            ident = wp.tile([P, P], dt.float32)
            make_identity(nc, ident[:])

            def bcast(t, reps):
                ap = t if not hasattr(t, 'tile_pool_tag') else t[:]
                try:
                    ap = t[:, None, :]
                except Exception:
                    ap = t[:][:, None, :]
                return ap.to_broadcast((P, reps, ap.shape[-1]))

            # ---------- V projections (cell-major bf16, bias fused on evac)
            fsrc = [f0, f1, f2]
            V_t = []
            for l in range(L_LEV):
                nkt, ncin = KT[l], fsrc[l].shape[2]
                vt = sp.tile([P, nkt, C], dt.bfloat16, tag=f"V{l}")
                for kt in range(nkt):
                    ft = fp.tile([P, 8, P], dt.bfloat16, tag="ftile")
                    nc.sync.dma_start(ft[:, :ncin], fsrc[l][:, kt])
                    ps = pp.tile([P, 512], dt.float32, tag="mm", name="ps")[:, :C]
                    for j in range(ncin):
                        nc.tensor.matmul(ps[:], ft[:, j], vwT_t[l][:, j],
                                         start=(j == 0), stop=(j == ncin - 1))
                    nc.vector.tensor_tensor(vt[:, kt], ps[:], vbrep_t[l],
                                            Alu.add)
                V_t.append(vt)

            # ---------- Q proj + LN (query-major f32)
            Qn = qp.tile([P, 8, C], dt.float32, tag="qact")
            for qt in range(8):
                st = tp.tile([P, 2, P], dt.float32, tag="stile")
                nc.sync.dma_start(st[:], s[:, :, qt * P:(qt + 1) * P])
                ps = pp.tile([P, 512], dt.float32, tag="mm", name="ps")[:, :C]
                for kt in range(2):
                    nc.tensor.matmul(ps[:], st[:, kt],
                                     qwT_t[:, kt], start=(kt == 0), stop=(kt == 1))
                nc.vector.tensor_tensor(Qn[:, qt], ps[:], qbrep_t, Alu.add)

            def layer_norm(X, g_t, b_t):
                mu = tp.tile([P, 8], dt.float32, tag="lnmu")
                m2 = tp.tile([P, 8], dt.float32, tag="lnm2")
                sq = tp.tile([P, C], dt.float32, tag="lnsq")
                for qt in range(8):
                    nc.vector.reduce_sum(mu[:, qt:qt + 1], X[:, qt],
                                         axis=mybir.AxisListType.X)
                    nc.scalar.activation(sq[:], X[:, qt], Act.Square)
                    nc.vector.reduce_sum(m2[:, qt:qt + 1], sq[:],
                                         axis=mybir.AxisListType.X)
                mean = tp.tile([P, 8], dt.float32, tag="lnmean")
                nc.vector.tensor_scalar(mean[:], mu[:], 1.0 / C, None, Alu.mult)
                var = tp.tile([P, 8], dt.float32, tag="lnvar")
                nc.vector.tensor_scalar(var[:], m2[:], 1.0 / C, None, Alu.mult)
                msq = tp.tile([P, 8], dt.float32, tag="lnmsq")
                nc.vector.tensor_tensor(msq[:], mean[:], mean[:], Alu.mult)
                nc.vector.tensor_tensor(var[:], var[:], msq[:], Alu.subtract)
                nc.vector.tensor_scalar(var[:], var[:], 1e-5, None, Alu.add)
                sd = tp.tile([P, 8], dt.float32, tag="lnsd")
                nc.scalar.activation(sd[:], var[:], Act.Sqrt)
                inv = tp.tile([P, 8], dt.float32, tag="lninv")
                nc.vector.reciprocal(inv[:], sd[:])
                for qt in range(8):
                    nc.vector.tensor_scalar(X[:, qt], X[:, qt],
                                            mean[:, qt:qt + 1], inv[:, qt:qt + 1],
                                            Alu.subtract, Alu.mult)
                nc.vector.tensor_tensor(X[:], X[:], bcast(g_t, 8), Alu.mult)
                nc.vector.tensor_tensor(X[:], X[:], bcast(b_t, 8), Alu.add)

            layer_norm(Qn, lnqg_t, lnqb_t)

            # ---------- Q^T (f32 PE transpose)
            QT = sp.tile([P, 16, P], dt.float32, tag="QT")
            for qt in range(8):
                for kt in range(2):
                    pst = pp.tile([P, 512], dt.float32, tag="mm", name="pst")[:, :P]
                    nc.tensor.transpose(pst[:], Qn[:, qt, kt * P:(kt + 1) * P],
                                        ident[:])
                    nc.vector.tensor_copy(QT[:, qt * 2 + kt], pst[:])

            # ---------- off/wgt matmul -> ow [P, 8, 144]
            ow = sp.tile([P, 8, 144], dt.float32, tag="ow")
            for qt in range(8):
                ps = pp.tile([P, 512], dt.float32, tag="mm", name="psow")[:, :144]
                for kt in range(2):
                    nc.tensor.matmul(ps[:], QT[:, qt * 2 + kt], owT_t[:, kt],
                                     start=(kt == 0), stop=(kt == 1))
                nc.vector.tensor_tensor(ow[:, qt], ps[:], owbrep_t, Alu.add)

            # ---------- tap math
            txy = sp.tile([P, 8, 96], dt.float32, tag="txy")
            for qt in range(8):
                nc.vector.tensor_scalar(txy[:, qt, 0:48], ow[:, qt, 0:48],
                                        refq_t[:, qt, 0:1], None, Alu.add)
                nc.vector.tensor_scalar(txy[:, qt, 48:96], ow[:, qt, 48:96],
                                        refq_t[:, qt, 1:2], None, Alu.add)
            nc.vector.tensor_scalar(txy[:], txy[:], 2.0, -1.0,
                                    Alu.mult, Alu.add)
            g = sp.tile([P, 8, 96], dt.float32, tag="g96")
            nc.scalar.activation(g[:], txy[:], Act.Tanh)
            p_ = txy
            nc.vector.tensor_tensor(p_[:], g[:], bcast(screp_t, 8), Alu.mult)
            nc.vector.tensor_tensor(p_[:], p_[:], bcast(screp_t, 8), Alu.add)
            pint = tp.tile([P, 8, 96], dt.int32, tag="w96a")
            nc.vector.tensor_copy(pint[:], p_[:])
            pff = g
            nc.vector.tensor_copy(pff[:], pint[:])
            dneg = tp.tile([P, 8, 96], dt.float32, tag="w96b")
            nc.vector.tensor_tensor(dneg[:], p_[:], pff[:], Alu.subtract)
            nc.vector.tensor_scalar(dneg[:], dneg[:], 0.0, None, Alu.is_lt)
            nc.vector.tensor_tensor(pff[:], pff[:], dneg[:], Alu.subtract)
            nc.vector.tensor_tensor(pff[:], pff[:], bcast(wm2rep_t, 8), Alu.min)
            frac = sp.tile([P, 8, 96], dt.float32, tag="frac")
            nc.vector.tensor_tensor(frac[:], p_[:], pff[:], Alu.subtract)
            idx00 = tp.tile([P, 8, 48], dt.float32, tag="w48a")
            nc.vector.tensor_tensor(idx00[:], pff[:, :, 48:96], bcast(wlrep_t, 8),
                                    Alu.mult)
            nc.vector.tensor_tensor(idx00[:], idx00[:], pff[:, :, 0:48], Alu.add)
            # softmax
            simp = tp.tile([P, 8], dt.float32, tag="simp")
            nc.vector.tensor_scalar(simp[:], simq_t[:], 0.001, None, Alu.add)
            e = sp.tile([P, 8, 48], dt.float32, tag="e48")
            for qt in range(8):
                nc.vector.tensor_scalar(e[:, qt], ow[:, qt, 96:144],
                                        simp[:, qt:qt + 1], None, Alu.mult)
            nc.scalar.activation(e[:], e[:], Act.Exp)
            den = tp.tile([P, 8, 4], dt.float32, tag="den")
            for h in range(4):
                nc.vector.reduce_sum(den[:, :, h:h + 1],
                                     e[:, :, h * 12:(h + 1) * 12],
                                     axis=mybir.AxisListType.X)
            dinv = tp.tile([P, 8, 4], dt.float32, tag="dinv")
            nc.vector.reciprocal(dinv[:], den[:])
            for qt in range(8):
                for h in range(4):
                    nc.vector.tensor_scalar(e[:, qt, h * 12:(h + 1) * 12],
                                            e[:, qt, h * 12:(h + 1) * 12],
                                            dinv[:, qt, h:h + 1], None, Alu.mult)
            # tap weights / indices  (tap t = m*4 + (dy*2+dx), per (h,l) 16)
            u = tp.tile([P, 8, 48], dt.float32, tag="w48b")
            v = tp.tile([P, 8, 48], dt.float32, tag="w48c")
            nc.vector.tensor_scalar(u[:], frac[:, :, 0:48], -1.0, 1.0,
                                    Alu.mult, Alu.add)
            nc.vector.tensor_scalar(v[:], frac[:, :, 48:96], -1.0, 1.0,
                                    Alu.mult, Alu.add)
            tw = sp.tile([P, 8, 12, 16], dt.float32, tag="txy")
            ti = sp.tile([P, 8, 12, 16], dt.float32, tag="g96")
            tw4 = tw[:].rearrange("p a b (m t) -> p a (b m) t", t=4)
            ti4 = ti[:].rearrange("p a b (m t) -> p a (b m) t", t=4)
            tmp48 = tp.tile([P, 8, 48], dt.float32, tag="w96b")
            combos = [(v, u), (v, frac[:, :, 0:48]),
                      (frac[:, :, 48:96], u), (frac[:, :, 48:96], frac[:, :, 0:48])]
            for t, (ya, xa) in enumerate(combos):
                ya = ya[:] if hasattr(ya, 'tensor') else ya
                xa = xa[:] if hasattr(xa, 'tensor') else xa
                nc.vector.tensor_tensor(tmp48[:], ya, xa, Alu.mult)
                nc.vector.tensor_tensor(tw4[:, :, :, t], tmp48[:], e[:], Alu.mult)
            nc.vector.tensor_copy(ti4[:, :, :, 0], idx00[:])
            nc.vector.tensor_scalar(ti4[:, :, :, 1], idx00[:], 1.0, None, Alu.add)
            nc.vector.tensor_tensor(tmp48[:], idx00[:], bcast(wlrep_t, 8), Alu.add)
            nc.vector.tensor_copy(ti4[:, :, :, 2], tmp48[:])
            nc.vector.tensor_scalar(ti4[:, :, :, 3], tmp48[:], 1.0, None, Alu.add)

            # ---------- duplicate-tap merge
            worig = sp.tile([P, 8, 12, 16], dt.float32, tag="QT")
            nc.vector.tensor_copy(worig[:], tw[:])
            seen = sp.tile([P, 8, 12, 16], dt.bfloat16, tag="frac")
            nc.vector.memset(seen[:], 0.0)

            def mj(t):
                return t[:].rearrange("p a b (m j) -> p a b m j", j=4)

            for d in range(1, 16):
                if d <= 3:
                    sl_a = (slice(0, 3), slice(4 - d, 4))
                    sl_b = (slice(1, 4), slice(0, d))
                    a_i = mj(ti)[:, :, :, sl_a[0], sl_a[1]]
                    b_i = mj(ti)[:, :, :, sl_b[0], sl_b[1]]
                    a_w = mj(worig)[:, :, :, sl_a[0], sl_a[1]]
                    b_w = mj(worig)[:, :, :, sl_b[0], sl_b[1]]
                    a_t = mj(tw)[:, :, :, sl_a[0], sl_a[1]]
                    b_t = mj(tw)[:, :, :, sl_b[0], sl_b[1]]
                    b_s = mj(seen)[:, :, :, sl_b[0], sl_b[1]]
                    eqt = tp.tile([P, 8, 12, 3, 3], dt.bfloat16,
                                  tag="w96a", name="eqt")[:, :, :, :, :d]
                    tm = tp.tile([P, 8, 12, 3, 3], dt.bfloat16,
                                 tag="w96b", name="tmt")[:, :, :, :, :d]
                else:
                    n = 16 - d
                    a_i = ti[:, :, :, 0:n]; b_i = ti[:, :, :, d:16]
                    a_w = worig[:, :, :, 0:n]; b_w = worig[:, :, :, d:16]
                    a_t = tw[:, :, :, 0:n]; b_t = tw[:, :, :, d:16]
                    b_s = seen[:, :, :, d:16]
                    eqt = tp.tile([P, 8, 12, 12], dt.bfloat16,
                                  tag="w96a", name="eqt2")[:, :, :, :n]
                    tm = tp.tile([P, 8, 12, 12], dt.bfloat16,
                                 tag="w96b", name="tmt2")[:, :, :, :n]
                nc.vector.tensor_tensor(eqt, a_i, b_i, Alu.is_equal)
                nc.vector.tensor_tensor(tm, eqt, b_w, Alu.mult)
                nc.vector.tensor_tensor(a_t, a_t, tm, Alu.add)
                nc.vector.tensor_tensor(tm, eqt, a_w, Alu.mult)
                nc.vector.tensor_tensor(b_t, b_t, tm, Alu.add)
                nc.vector.tensor_tensor(b_s, b_s, eqt, Alu.max)

            om = worig
            nc.vector.tensor_scalar(om[:], seen[:], -1.0, 1.0, Alu.mult, Alu.add)
            nc.vector.tensor_tensor(ti[:], ti[:], om[:], Alu.mult)
            nc.vector.tensor_tensor(ti[:], ti[:], seen[:], Alu.subtract)

            # casts
            wbf = sp.tile([P, 8, 12, 16], dt.bfloat16, tag="ow")
            nc.vector.tensor_copy(wbf[:], tw[:])
            i16 = sp.tile([P, 8, 12, 16], dt.int16, tag="frac")
            nc.vector.tensor_copy(i16[:], ti[:])
            ti_l0 = ti[:].rearrange("p a (h l) t -> p a h l t", l=3)[:, :, :, 0]
            i16c = sp.tile([P, 8, 4, 4, 16], dt.int16, tag="e48")
            for ch in range(4):
                tch = tp.tile([P, 8, 4, 16], dt.float32, tag="lnsq")
                nc.vector.tensor_scalar(tch[:], ti_l0, float(ch * 1024), None,
                                        Alu.subtract)
                m1 = tp.tile([P, 8, 4, 16], dt.bfloat16, tag="l0b")
                nc.vector.tensor_scalar(m1[:], tch[:], 0.0, None, Alu.is_ge)
                m2 = tp.tile([P, 8, 4, 16], dt.bfloat16, tag="l0c")
                nc.vector.tensor_scalar(m2[:], tch[:], 1024.0, None, Alu.is_lt)
                nc.vector.tensor_tensor(m1[:], m1[:], m2[:], Alu.mult)
                nc.vector.tensor_scalar(tch[:], tch[:], 1.0, None, Alu.add)
                nc.vector.tensor_tensor(tch[:], tch[:], m1[:], Alu.mult)
                nc.vector.tensor_scalar(tch[:], tch[:], 1.0, None, Alu.subtract)
                nc.vector.tensor_copy(i16c[:, :, ch], tch[:])

            # ---------- sampling (per head, per query-half of 512)
            accB = mp.tile([P, 8, NQ], dt.bfloat16, tag="accB")
            for h in range(4):
                psq = [[pp.tile([P, 512], dt.float32, tag="mm", name="psq")
                        for _ in range(2)] for _ in range(2)]
                first = [[True, True], [True, True]]

                def mms(lev, At, jlist, half, stop_at=None):
                    if 'mm' in SKIP:
                        jlist = [jlist[0], jlist[-1]] if stop_at else [jlist[0]]
                    for jj, ktg in jlist:
                        for mt in range(2):
                            st = (stop_at == (jj, ktg, mt + 1))
                            nc.tensor.matmul(
                                psq[mt][half][:],
                                V_t[lev][:, ktg, mt * P:(mt + 1) * P],
                                At[:, jj, :],
                                start=first[mt][half], stop=st)
                            first[mt][half] = False

                for half in range(2):
                    # L0
                    A0 = ap_.tile([P, 4, 4096], dt.bfloat16, tag="A")
                    for q2 in range(4):
                        qt = half * 4 + q2
                        for ch in range(4):
                            if 'scat' in SKIP: continue
                            nc.gpsimd.local_scatter(
                                A0[:, q2, ch * 1024:(ch + 1) * 1024],
                                wbf[:, qt, h * 3 + 0], i16c[:, qt, ch, h],
                                channels=P, num_elems=1024, num_idxs=16)
                    for ch in range(4):
                        At = atp.tile([P, 8, 512], dt.bfloat16, tag="At")
                        nc.gpsimd.dma_gather(
                            At[:], A0[:], gidx_t[:],
                            num_idxs=512, num_idxs_reg=512,
                            elem_size=1024, transpose=True,
                            sbuf_tokens_per_rank=P,
                            sbuf_free_dim_per_rank=4096 * 2,
                            sbuf_free_dim_pad_per_rank=0,
                            sbuf_byte_offset=ch * 2048,
                        )
                        mms(0, At, [(j, ch * 8 + j) for j in range(8)], half)
                    # L1 + L2 concat
                    A12 = ap_.tile([P, 4, 1280], dt.bfloat16, tag="A")
                    for q2 in range(4):
                        qt = half * 4 + q2
                        nc.gpsimd.local_scatter(
                            A12[:, q2, 0:1024],
                            wbf[:, qt, h * 3 + 1], i16[:, qt, h * 3 + 1],
                            channels=P, num_elems=1024, num_idxs=16)
                        nc.gpsimd.local_scatter(
                            A12[:, q2, 1024:1280],
                            wbf[:, qt, h * 3 + 2], i16[:, qt, h * 3 + 2],
                            channels=P, num_elems=256, num_idxs=16)
                    At = atp.tile([P, 8, 512], dt.bfloat16, tag="At")
                    nc.gpsimd.dma_gather(
                        At[:, :5], A12[:], gidx_t[:],
                        num_idxs=512, num_idxs_reg=512,
                        elem_size=640, transpose=True,
                        sbuf_tokens_per_rank=P,
                        sbuf_free_dim_per_rank=1280 * 2,
                        sbuf_free_dim_pad_per_rank=0,
                        sbuf_byte_offset=0,
                    )
                    mms(1, At, [(j, j) for j in range(5)], half)
                    At2 = atp.tile([P, 8, 512], dt.bfloat16, tag="At")
                    nc.gpsimd.dma_gather(
                        At2[:, :5], A12[:], gidx_t[:],
                        num_idxs=512, num_idxs_reg=512,
                        elem_size=640, transpose=True,
                        sbuf_tokens_per_rank=P,
                        sbuf_free_dim_per_rank=1280 * 2,
                        sbuf_free_dim_pad_per_rank=0,
                        sbuf_byte_offset=1280,
                    )
                    mms(1, At2, [(j, 5 + j) for j in range(3)], half)
                    mms(2, At2[:, 3:], [(j, j) for j in range(2)], half,
                        stop_at=(1, 1, 2))
                for mt in range(2):
                    for half in range(2):
                        nc.vector.tensor_copy(
                            accB[:, h * 2 + mt, half * 512:(half + 1) * 512],
                            psq[mt][half][:])

            # ---------- late weights into a freed A slot
            lb_t = ap_.tile([P, 6144], dt.bfloat16, tag="A", name="lb")
            nc.sync.dma_start(lb_t[:], lblob[:])
            outwT_t = lb_t[:, 0:2048].rearrange("p (k o) -> p k o", k=8)
            fc1T_t = lb_t[:, 2048:4096].rearrange("p (k o) -> p k o", k=2)
            fc2T_t = lb_t[:, 4096:6144].rearrange("p (k o) -> p k o", k=8)

            # ---------- out_proj + residual + LN
            Zr = qp.tile([P, 8, C], dt.float32, tag="qact")
            for qt in range(8):
                ps = pp.tile([P, 512], dt.float32, tag="mm", name="ps")[:, :C]
                for kt in range(8):
                    nc.tensor.matmul(ps[:], accB[:, kt, qt * P:(qt + 1) * P],
                                     outwT_t[:, kt], start=(kt == 0),
                                     stop=(kt == 7))
                stt = tp.tile([P, C], dt.float32, tag="sttile")
                nc.sync.dma_start(stt[:], sT[:, qt])
                nc.vector.tensor_tensor(Zr[:, qt], ps[:], outbrep_t, Alu.add)
                nc.vector.tensor_tensor(Zr[:, qt], Zr[:, qt], stt[:],
                                        Alu.add)
            layer_norm(Zr, lnog_t, lnob_t)

            # ---------- FFN
            identb = tp.tile([P, P], dt.bfloat16, tag="stile")
            nc.vector.tensor_copy(identb[:], ident[:])
            Zb = sp.tile([P, 8, C], dt.bfloat16, tag="e48")
            nc.vector.tensor_copy(Zb[:], Zr[:])
            ZT = sp.tile([P, 16, P], dt.bfloat16, tag="QT")
            for qt in range(8):
                for kt in range(2):
                    pst = pp.tile([P, P], dt.bfloat16, tag="mm", name="pstb")
                    nc.tensor.transpose(pst[:], Zb[:, qt, kt * P:(kt + 1) * P],
                                        identb[:])
                    nc.vector.tensor_copy(ZT[:, qt * 2 + kt], pst[:])
            h1F = mp.tile([P, 8, NQ], dt.bfloat16, tag="accB")
            for nch in range(2):
                pss = [pp.tile([P, 512], dt.float32, tag="mm", name="pss") for _ in range(8)]
                for mt in range(8):
                    for kt in range(2):
                        nc.tensor.matmul(
                            pss[mt][:], fc1T_t[:, kt, mt * P:(mt + 1) * P],
                            ZT[:, nch * 8 + kt:nch * 8 + kt + 7:2],
                            start=(kt == 0), stop=(kt == 1))
                    nc.scalar.activation(h1F[:, mt, nch * 512:(nch + 1) * 512],
                                         pss[mt][:], Act.Gelu,
                                         bias=fc1b_t[:, mt:mt + 1])
            out_s = qp.tile([P, 8, C], dt.float32, tag="qact")
            for qt in range(8):
                ps = pp.tile([P, 512], dt.float32, tag="mm", name="ps")[:, :C]
                for kt in range(8):
                    nc.tensor.matmul(ps[:], h1F[:, kt, qt * P:(qt + 1) * P],
                                     fc2T_t[:, kt], start=(kt == 0),
                                     stop=(kt == 7))
                nc.vector.tensor_tensor(out_s[:, qt], ps[:], fc2brep_t,
                                        Alu.add)
                nc.vector.tensor_tensor(out_s[:, qt], out_s[:, qt], Zr[:, qt],
                                        Alu.add)
            nc.sync.dma_start(out[:], out_s[:])
    nc.finalize()
    return nc


def _prep_inputs(inputs):
    bf = ml_dtypes.bfloat16
    S = np.ascontiguousarray(inputs['S'], dtype=np.float32)
    f0 = np.ascontiguousarray(inputs['f0'], dtype=np.float32)
    f1 = np.ascontiguousarray(inputs['f1'], dtype=np.float32)
    f2 = np.ascontiguousarray(inputs['f2'], dtype=np.float32)
    sim = np.ascontiguousarray(inputs['sim'], dtype=np.float32)

    def wT_tiled(w, dty=np.float32):
        w = np.asarray(w, np.float32)
        cin = w.shape[1]
        return np.ascontiguousarray(
            w.T.reshape(cin // P, P, w.shape[0]).transpose(1, 0, 2)).astype(dty)

    def rep(x):
        x = np.asarray(x, np.float32)
        return np.ascontiguousarray(np.broadcast_to(x[None, :], (P, len(x))))

    off_w = np.asarray(inputs['off_w'], np.float32)
    wgt_w = np.asarray(inputs['wgt_w'], np.float32)
    off_b = np.asarray(inputs['off_b'], np.float32)
    xrows = [((h * L_LEV + l) * M_PTS + m) * 2
             for h in range(4) for l in range(3) for m in range(4)]
    OW = np.concatenate([off_w[xrows], off_w[[r + 1 for r in xrows]], wgt_w], 0)
    OWb = np.concatenate([off_b[xrows], off_b[[r + 1 for r in xrows]],
                          np.asarray(inputs['wgt_b'], np.float32)])

    lev = (np.arange(48) // M_PTS) % L_LEV
    dims = np.array([DIMS[l] for l in lev], np.float32)
    screp = rep(np.concatenate([(dims - 1) / 2, (dims - 1) / 2]))
    wm2rep = rep(np.concatenate([dims - 2, dims - 2]))
    wlrep = rep(dims)

    gidx = np.zeros((16, 32), dtype=np.int16)
    for j in range(512):
        gidx[j % 16, j // 16] = j
    gidx = np.tile(gidx, (8, 1))

    def flat(a):
        return np.asarray(a, dtype=None).reshape(P, -1)
    bblob = np.concatenate([
        flat(wT_tiled(inputs['vw0'], bf)), flat(wT_tiled(inputs['vw1'], bf)),
        flat(wT_tiled(inputs['vw2'], bf)),
    ], axis=1).astype(bf)
    lblob = np.concatenate([
        flat(wT_tiled(inputs['out_w'], bf)),
        flat(wT_tiled(inputs['fc1_w'], bf)), flat(wT_tiled(inputs['fc2_w'], bf)),
    ], axis=1).astype(bf)
    fc1b_h = np.ascontiguousarray(
        np.asarray(inputs['fc1_b'], np.float32).reshape(8, P).T)
    fconst = np.concatenate([
        flat(wT_tiled(inputs['q_w'])), flat(rep(inputs['q_b'])),
        flat(wT_tiled(OW)), flat(rep(OWb)),
        flat(rep(inputs['vb0'])), flat(rep(inputs['vb1'])),
        flat(rep(inputs['vb2'])), flat(rep(inputs['out_b'])),
        flat(fc1b_h), flat(rep(inputs['fc2_b'])),
        flat(rep(inputs['lnq_g'])), flat(rep(inputs['lnq_b'])),
        flat(rep(inputs['lno_g'])), flat(rep(inputs['lno_b'])),
        flat(screp), flat(wm2rep), flat(wlrep),
    ], axis=1).astype(np.float32)
    shared = {'bblob': np.ascontiguousarray(bblob), 'lblob': np.ascontiguousarray(lblob), 'gidx': gidx}

    def ftile(f, b, nkt, nht):
        a = f[b].reshape(nkt, P, nht, P)
        return np.ascontiguousarray(a.transpose(1, 2, 0, 3)).astype(bf)

    in_maps = []
    for c in range(N_CORES):
        b, q4 = c // 4, c % 4
        qsl = slice(q4 * NQ, (q4 + 1) * NQ)
        Sb = S[b].reshape(C, 4096)
        s_ = np.ascontiguousarray(Sb[:, qsl].reshape(2, P, NQ).transpose(1, 0, 2))
        sT_ = np.ascontiguousarray(Sb[:, qsl].T.reshape(8, P, C).transpose(1, 0, 2))
        qg = np.arange(q4 * NQ, (q4 + 1) * NQ)
        refx = ((qg % 64 + 0.5) / 64).astype(np.float32)
        refy = ((qg // 64 + 0.5) / 64).astype(np.float32)
        refq = np.stack([refx.reshape(8, P).T, refy.reshape(8, P).T],
                        axis=-1).astype(np.float32)
        simq = np.ascontiguousarray(sim[b, 0].reshape(4096)[qsl].reshape(8, P).T)
        fblob = np.ascontiguousarray(np.concatenate(
            [flat(simq), flat(refq), fconst], axis=1).astype(np.float32))
        in_maps.append({
            's': s_, 'sT': sT_,
            'f0': ftile(f0, b, 2, 32), 'f1': ftile(f1, b, 4, 8),
            'f2': ftile(f2, b, 8, 2),
            'fblob': fblob,
            **shared,
        })
    return in_maps


def _get_runner():
    if 'runner' not in _CACHE:
        import jax
        from jax.sharding import Mesh, PartitionSpec
        from jax.experimental.shard_map import shard_map
        import concourse.mybir as mybir
        from concourse import bass2jax

        nc = _build()
        bass2jax.install_neuronx_cc_hook()
        partition_name = (
            nc.partition_id_tensor.name if nc.partition_id_tensor else None)
        in_names, out_names, out_avals, zero_outs = [], [], [], []
        for alloc in nc.m.functions[0].allocations:
            if not isinstance(alloc, mybir.MemoryLocationSet):
                continue
            name = alloc.memorylocations[0].name
            if alloc.kind == "ExternalInput":
                if name != partition_name:
                    in_names.append(name)
            elif alloc.kind == "ExternalOutput":
                shape = tuple(alloc.tensor_shape)
                dtype = mybir.dt.np(alloc.dtype)
                out_names.append(name)
                out_avals.append(jax.core.ShapedArray(shape, dtype))
                zero_outs.append(np.zeros(shape, dtype))
        all_names = list(in_names) + list(out_names)
        if partition_name is not None:
            all_names.append(partition_name)

        def _body(*args):
            operands = list(args)
            if partition_name is not None:
                operands.append(bass2jax.partition_id_tensor())
            outs = bass2jax._bass_exec_p.bind(
                *operands,
                out_avals=tuple(out_avals),
                in_names=tuple(all_names),
                out_names=tuple(out_names),
                lowering_input_output_aliases=(),
                sim_require_finite=True,
                sim_require_nnan=True,
                nc=nc,
            )
            return tuple(outs)

        devices = jax.devices()[:N_CORES]
        mesh = Mesh(np.asarray(devices), ("core",))
        n_params, n_outs = len(in_names), len(out_names)
        fn = jax.jit(
            shard_map(_body, mesh=mesh,
                      in_specs=(PartitionSpec("core"),) * (n_params + n_outs),
                      out_specs=(PartitionSpec("core"),) * n_outs,
                      check_rep=False),
            keep_unused=True)
        _CACHE['runner'] = (fn, in_names, out_names, out_avals, zero_outs)
    return _CACHE['runner']


def run_cores(in_maps):
    import jax
    fn, in_names, out_names, out_avals, zero_outs = _get_runner()
    concat_in = [
        np.concatenate([np.asarray(m[name]) for m in in_maps], axis=0)
        for name in in_names
    ]
    concat_zeros = [
        np.zeros((N_CORES * z.shape[0], *z.shape[1:]), z.dtype)
        for z in zero_outs
    ]
    out_arrs = jax.block_until_ready(fn(*(concat_in + concat_zeros)))
    return [
        {name: np.asarray(out_arrs[i]).reshape(N_CORES, *out_avals[i].shape)[c]
         for i, name in enumerate(out_names)}
        for c in range(N_CORES)
    ]


def kernel(**inputs) -> np.ndarray:
    in_maps = _prep_inputs(inputs)
    res = run_cores(in_maps)
    out = np.zeros((2, C, 64, 64), dtype=np.float32)
    for c in range(N_CORES):
        b, q4 = c // 4, c % 4
        z = res[c]['out']
        zq = z.transpose(1, 0, 2).reshape(NQ, C)
        out[b].reshape(C, 4096)[:, q4 * NQ:(q4 + 1) * NQ] = zq.T
    return out

